# revision 9
# baseline (speedup 1.0000x reference)
"""Grouped-experts SwiGLU FFN on 8 TRN2 NeuronCores.

Per-expert computation: out_e = (silu(x_e @ w1_e) * (x_e @ w3_e)) @ w2_e
with E=8, T=2048, D=2048, H=4096 (fp32).

Sharding: expert-parallel — core e owns expert e (x[e], w1[e], w2[e], w3[e]);
no cross-core communication is needed since the per-expert outputs are
independent.

Per-core kernel (Tile framework):
  Phase 0: transpose x [T,D] -> xT [D,T] via PE-transpose (128x128 blocks),
           xT kept SBUF-resident as two 64KB/partition tiles.
  Phase A: hT = w1.T @ xT-chunks, accumulate over D in PSUM (float32r
           matmuls: full-rate fp32 on the PE for moving dim >= 256);
           g = silu(h1) * h3 fused on ACT/DVE with a bf16 downcast,
           bounced to an internal DRAM buffer gT [H,T] (bf16).
  Phase B: out = g @ w2, contraction over H: lhsT = gT tiles (bf16),
           rhs = w2 tiles cast fp32->bf16 on-chip, PSUM accumulate over
           all 32 k-tiles, evict to out [T,D].
"""

import os
import sys
from contextlib import ExitStack

import numpy as np

for _p in ("/opt/trn_rl_repo", "/root/.axon_site/_ro/trn_rl_repo"):
    if os.path.isdir(_p) and _p not in sys.path:
        sys.path.insert(0, _p)

import concourse.bass as bass
import concourse.tile as tile
from concourse import bacc, mybir
from concourse._compat import with_exitstack
from concourse.bass_utils import run_bass_kernel_spmd
from concourse.masks import make_identity

E, T, D, H = 8, 2048, 2048, 4096
P = 128
KD = D // P        # 16 k-tiles over D (mm1/mm3 contraction)
KH = H // P        # 32 k-tiles over H (mm2 contraction)
HM = H // P        # 32 output-partition tiles of hT
TN = T // 512      # 4 moving chunks of T for mm1/mm3
TM = T // P        # 16 output-partition tiles of out
DB = 256           # mm2 moving-dim chunk of D
DN = D // DB       # 8

F32 = mybir.dt.float32
F32R = mybir.dt.float32r
BF16 = mybir.dt.bfloat16
SIGMOID = mybir.ActivationFunctionType.Sigmoid

TRACE = False
LAST_RESULTS = None
_CACHED_NC = None


@with_exitstack
def _swiglu_body(ctx: ExitStack, tc: "tile.TileContext", out, x, w1, w2, w3, gT):
    nc = tc.nc

    consts = ctx.enter_context(tc.tile_pool(name="consts", bufs=1))
    big = ctx.enter_context(tc.tile_pool(name="big", bufs=2))
    psum = ctx.enter_context(tc.tile_pool(name="psum", bufs=8, space="PSUM"))

    ident = consts.tile([P, P], F32)
    make_identity(nc, ident[:])

    # xT halves: [P, 8, T] fp32 = 64KB/partition each. Tag shared with the
    # phase-B gT halves (same slot size) so phase B reuses the memory.
    xt_lo = big.tile([P, KD // 2, T], F32R, tag="big")
    xt_hi = big.tile([P, KD // 2, T], F32R, tag="big")

    def xT(k):
        return (xt_lo if k < KD // 2 else xt_hi)[:, k % (KD // 2), :]

    with tc.tile_pool(name="w13", bufs=6) as w13, \
         tc.tile_pool(name="xstage", bufs=4) as xstage, \
         tc.tile_pool(name="smallA", bufs=4) as smallA:
        # ---- Phase 0: transpose x into xT (t-chunk-major so phase A can
        # start after the first T-chunk's column tiles are ready).
        for tn in range(TN):
            for tt in range(4):          # four 128-rows of this 512-chunk
                t = tn * 4 + tt
                for k in range(KD):
                    xt = xstage.tile([P, P], F32, tag="xs")
                    nc.sync.dma_start(
                        xt[:], x[t * P:(t + 1) * P, k * P:(k + 1) * P]
                    )
                    ps = psum.tile([P, P], F32, tag="ps")
                    nc.tensor.transpose(ps[:], xt[:], ident[:])
                    nc.vector.tensor_copy(
                        xT(k)[:, t * P:(t + 1) * P], ps[:]
                    )

        # ---- Phase A: hT tiles [128(H), 512(T)] = silu(w1.T @ xT) * (w3.T @ xT)
        w1r = w1.rearrange("(k p) h -> p k h", p=P)
        w3r = w3.rearrange("(k p) h -> p k h", p=P)
        for hm in range(HM):
            w1blk = w13.tile([P, KD, P], F32R, tag="w13")
            w3blk = w13.tile([P, KD, P], F32R, tag="w13")
            nc.sync.dma_start(
                w1blk[:], w1r[:, :, hm * P:(hm + 1) * P].bitcast(F32R)
            )
            nc.sync.dma_start(
                w3blk[:], w3r[:, :, hm * P:(hm + 1) * P].bitcast(F32R)
            )
            for tn in range(TN):
                ts_ = slice(tn * 512, (tn + 1) * 512)
                ps1 = psum.tile([P, 512], F32, tag="ps")
                ps3 = psum.tile([P, 512], F32, tag="ps")
                for k in range(KD):
                    nc.tensor.matmul(
                        ps1[:],
                        w1blk[:, k, :],
                        xT(k)[:, ts_],
                        start=(k == 0),
                        stop=(k == KD - 1),
                    )
                for k in range(KD):
                    nc.tensor.matmul(
                        ps3[:],
                        w3blk[:, k, :],
                        xT(k)[:, ts_],
                        start=(k == 0),
                        stop=(k == KD - 1),
                    )
                # silu(h1)*h3 = h1*sigmoid(h1)*h3; each DVE op reads at
                # most one PSUM operand (verifier NCC_IBVF027).
                sig = smallA.tile([P, 512], F32, tag="sig")
                nc.scalar.activation(sig[:], ps1[:], SIGMOID)
                prod = smallA.tile([P, 512], F32, tag="prod")
                nc.vector.tensor_mul(prod[:], sig[:], ps3[:])
                g = smallA.tile([P, 512], BF16, tag="g")
                nc.vector.tensor_mul(g[:], prod[:], ps1[:])
                nc.sync.dma_start(gT[hm * P:(hm + 1) * P, ts_], g[:])

    # ---- Phase B: out[T,D] = g @ w2, k over H (32 tiles).
    with tc.tile_pool(name="w2stage", bufs=2) as w2stage, \
         tc.tile_pool(name="w2bf", bufs=2) as w2bf, \
         tc.tile_pool(name="oevict", bufs=8) as oevict:
        # gT halves reuse the xT slots (released above).
        g_lo = big.tile([P, KH // 2, T], BF16, tag="big")
        g_hi = big.tile([P, KH // 2, T], BF16, tag="big")

        def gblk(k):
            return (g_lo if k < KH // 2 else g_hi)[:, k % (KH // 2), :]

        # Per-k strip DMAs so phase-B matmuls can start as strips land.
        for k in range(KH):
            dst = (g_lo if k < KH // 2 else g_hi)
            nc.sync.dma_start(
                dst[:, k % (KH // 2), :], gT[k * P:(k + 1) * P, :]
            )

        for dn in range(DN):
            ds_ = slice(dn * DB, (dn + 1) * DB)
            wblk = w2bf.tile([P, KH, DB], BF16, tag="w2bf")
            for half in range(2):
                wst = w2stage.tile([P, KH // 2, DB], F32, tag="w2s")
                ks = half * (KH // 2)
                nc.sync.dma_start(
                    wst[:],
                    w2.rearrange("(k p) d -> p k d", p=P)[
                        :, ks:ks + KH // 2, ds_
                    ],
                )
                nc.scalar.copy(wblk[:, ks:ks + KH // 2, :], wst[:])
            for tg in range(4):          # groups of 4 output-partition tiles
                pss = [
                    psum.tile([P, DB], F32, tag="ps", name=f"pso_{dn}_{tg}_{i}")
                    for i in range(4)
                ]
                for k in range(KH):
                    for i in range(4):
                        tm = tg * 4 + i
                        nc.tensor.matmul(
                            pss[i][:],
                            gblk(k)[:, tm * P:(tm + 1) * P],
                            wblk[:, k, :],
                            start=(k == 0),
                            stop=(k == KH - 1),
                        )
                for i in range(4):
                    tm = tg * 4 + i
                    ev = oevict.tile([P, DB], F32, tag="ev")
                    nc.vector.tensor_copy(ev[:], pss[i][:])
                    nc.sync.dma_start(
                        out[tm * P:(tm + 1) * P, ds_], ev[:]
                    )


def _build():
    nc = bacc.Bacc("TRN2", debug=False, num_devices=E)
    x = nc.dram_tensor("x", (T, D), F32, kind="ExternalInput").ap()
    w1 = nc.dram_tensor("w1", (D, H), F32, kind="ExternalInput").ap()
    w2 = nc.dram_tensor("w2", (H, D), F32, kind="ExternalInput").ap()
    w3 = nc.dram_tensor("w3", (D, H), F32, kind="ExternalInput").ap()
    out = nc.dram_tensor("out", (T, D), F32, kind="ExternalOutput").ap()
    gT = nc.dram_tensor("gT", (H, T), BF16, kind="Internal").ap()
    with tile.TileContext(nc) as tc:
        _swiglu_body(tc, out, x, w1, w2, w3, gT)
    nc.compile()
    return nc


def _get_nc():
    global _CACHED_NC
    if _CACHED_NC is None:
        _CACHED_NC = _build()
    return _CACHED_NC


def kernel(x, w1, w2, w3):
    global LAST_RESULTS
    x = np.ascontiguousarray(np.asarray(x, dtype=np.float32))
    w1 = np.ascontiguousarray(np.asarray(w1, dtype=np.float32))
    w2 = np.ascontiguousarray(np.asarray(w2, dtype=np.float32))
    w3 = np.ascontiguousarray(np.asarray(w3, dtype=np.float32))
    assert x.shape == (E, T, D), x.shape

    nc = _get_nc()
    in_maps = [
        {"x": x[e], "w1": w1[e], "w2": w2[e], "w3": w3[e]} for e in range(E)
    ]
    res = run_bass_kernel_spmd(
        nc, in_maps, core_ids=list(range(E)), trace=TRACE
    )
    LAST_RESULTS = res
    return np.stack([res.results[e]["out"] for e in range(E)], axis=0)
